# revision 7
# baseline (speedup 1.0000x reference)
"""Trainium2 Bass kernel for nn_ContrastiveLoss (SimCLR-style, N=8192, D=128).

Sharding: rows of the NxN sim matrix split across 8 cores (1024 rows each).
Each core receives the full z = concat(emb0, emb1) ROTATED so its own rows
come first (np.roll(z, -core*1024, axis=0)).  With that rotation the positive
pair of local row l is local row l+4096 on every core -> one SPMD program,
no collectives.

Math (per row r, fixed max = 1.0 since cosine sim <= 1):
  S_r    = sum_j exp(10*G_rj - 10) - exp(10*G_rr - 10)   [G_rr = 1 => subtract 1]
  loss_r = (10 + ln S_r) - 10*G_pos
  loss   = mean_r(loss_r);  per-core output = [128,1] partial sums of loss_r.

Engine split (ACT is the bottleneck; everything else hides under it):
  ACT : one natural_log_exp table load, exp(psum chunks) with accum_out row
        sums, one tiny Ln at the end.  No sqrt -> no table thrash.
  PE  : only the G-matmuls (bf16).  Transposes moved to the DMA xbar.
  DVE : norms via mul+reduce + Newton rsqrt (fast-inverse-sqrt bit trick),
        normalize to bf16, positive-pair dot products, epilogue.
  DMA : input load + zn -> znT transposes via dma_start_transpose.

Prologue is latency-critical: tiles 0-15 are loaded/normalized/transposed in
4-tile slices and row-block 0's first column chunks are 512 wide, so the exp
stream starts as soon as the first slice is ready.  tile_wait_until() keeps
the greedy Tile scheduler from interleaving later groups' big DVE ops into
the critical chain.
"""

import sys

sys.path.insert(0, "/opt/trn_rl_repo")

from contextlib import ExitStack

import numpy as np

import concourse.bass as bass
import concourse.bacc as bacc
import concourse.tile as tile
from concourse import mybir
from concourse import bass_utils

B = 4096
D = 128
N = 2 * B            # 8192 rows of z
NCORES = 8
ROWS = N // NCORES   # 1024 rows per core
NBLK = ROWS // 128   # 8 row-blocks per core
CHUNK = 2048         # psum tile width (4 banks)
NCHUNK = N // CHUNK  # 4 column chunks
SEG = 512            # matmul moving-operand width
NTILE = N // 128     # 64 partition-tiles of z
GRP = 16             # tiles per DMA / norm group (groups 1..3)
SUB = 4              # tiles per slice in group 0
INV_T = 10.0         # 1/temperature
MAGIC = 0x5F3759DF

F32 = mybir.dt.float32
BF16 = mybir.dt.bfloat16
U32 = mybir.dt.uint32
AX = mybir.AxisListType
AF = mybir.ActivationFunctionType
ALU = mybir.AluOpType

# per-row-block column-chunk widths (in tiles of 128): block 0 starts fine-
# grained so the first exp only needs 4 transposed tiles.
CHUNKS_B0 = [4, 4, 4, 4, 16, 16, 16]
CHUNKS_BN = [16, 16, 16, 16]
MAXC = 7


def _build() -> bass.Bass:
    nc = bacc.Bacc(None)
    z_in = nc.declare_dram_parameter("z", [N, D], F32, isOutput=False)
    out = nc.declare_dram_parameter("partial", [128, 1], F32, isOutput=True)

    z_re = z_in.rearrange("(n p) d -> p n d", p=128)  # row = n*128 + p

    with tile.TileContext(nc) as tc:
        with ExitStack() as ctx:
            persist = ctx.enter_context(tc.tile_pool(name="persist", bufs=1))
            junkp = ctx.enter_context(tc.tile_pool(name="junk", bufs=2))
            psum = ctx.enter_context(tc.tile_pool(name="psum", bufs=2, space="PSUM"))

            # non-Copy activations need bias as an SBUF AP
            b_zero = persist.tile([128, 1], F32)
            nc.vector.memset(b_zero, 0.0)
            b_neg10 = persist.tile([128, 1], F32)
            nc.vector.memset(b_neg10, -INV_T)
            magic = persist.tile([128, 1], U32)
            nc.vector.memset(magic, MAGIC)

            z_sb = persist.tile([128, NTILE, D], F32)
            sq = persist.tile([128, NTILE, D], F32)
            ss = persist.tile([128, NTILE], F32)       # sumsq -> clamped
            rn = persist.tile([128, NTILE], F32)       # 1/norm (Newton rsqrt)
            nt0 = persist.tile([128, NTILE], F32)      # newton temps
            nt1 = persist.tile([128, NTILE], F32)
            zn_all = persist.tile([128, NTILE, D], BF16)
            znT = persist.tile([128, NTILE, D], BF16)  # [d, tile, row%128]
            acc = persist.tile([128, NBLK, MAXC], F32)     # per-chunk exp sums
            pprod = persist.tile([128, NBLK, D], F32)      # pos-pair products
            gpos = persist.tile([128, NBLK], F32)          # G_pos per row

            nc.vector.memset(acc, 0.0)

            def load_tiles(t0, t1):
                sl = slice(t0, t1)
                nc.sync.dma_start(out=z_sb[:, sl, :], in_=z_re[:, sl, :])

            def norm_tiles(t0, t1):
                sl = slice(t0, t1)
                w = t1 - t0
                nc.vector.tensor_mul(sq[:, sl, :], z_sb[:, sl, :], z_sb[:, sl, :])
                nc.vector.reduce_sum(ss[:, sl], sq[:, sl, :], axis=AX.X)
                nc.vector.tensor_scalar_max(ss[:, sl], ss[:, sl], 1e-16)
                # y0 = bitcast(MAGIC - (bits(s) >> 1)); two Newton steps
                # (DVE int add saturates, so subtract from a const tile)
                s_u = ss[:, sl].bitcast(U32)
                y_u = rn[:, sl].bitcast(U32)
                t_u = nt0[:, sl].bitcast(U32)
                nc.vector.tensor_scalar(t_u, s_u, 1, None, ALU.logical_shift_right)
                nc.vector.tensor_sub(y_u, magic.broadcast_to((128, w)), t_u)
                for _ in range(2):
                    nc.vector.tensor_mul(nt0[:, sl], rn[:, sl], rn[:, sl])
                    nc.vector.tensor_mul(nt1[:, sl], nt0[:, sl], ss[:, sl])
                    nc.vector.tensor_scalar(
                        nt0[:, sl], nt1[:, sl], -0.5, 1.5, ALU.mult, ALU.add
                    )
                    nc.vector.tensor_mul(rn[:, sl], rn[:, sl], nt0[:, sl])
                nc.vector.tensor_mul(
                    zn_all[:, sl, :],
                    z_sb[:, sl, :],
                    rn[:, sl].broadcast_to((128, w, D)),
                )

            def transpose_tiles(t0, t1):
                sl = slice(t0, t1)
                nc.sync.dma_start_transpose(znT[:, sl, :], zn_all[:, sl, :])

            def emit_chunk(b, c, ct0, ctw):
                # G chunk: rows of block b vs column tiles [ct0, ct0+ctw)
                lhsT = znT[:, b, :]  # [128(d), 128 rows]
                pt = psum.tile([128, ctw * D], F32, tag="pp", name="pt")
                for s in range(ctw * D // SEG):
                    cs = ct0 + s * (SEG // D)
                    nc.tensor.matmul(
                        pt[:, s * SEG : (s + 1) * SEG],
                        lhsT,
                        znT[:, cs : cs + SEG // D, :],
                        start=True,
                        stop=True,
                    )
                ej = junkp.tile([128, ctw * D], BF16, tag="ej", name="ej")
                nc.scalar.activation(
                    ej, pt, AF.Exp, scale=INV_T, bias=b_neg10,
                    accum_out=acc[:, b, c : c + 1],
                )

            # ---- prologue --------------------------------------------------
            for s in range(4):
                with tc.tile_wait_until(0.0012 * s, enable=s > 0):
                    load_tiles(s * SUB, (s + 1) * SUB)
                    norm_tiles(s * SUB, (s + 1) * SUB)
                    transpose_tiles(s * SUB, (s + 1) * SUB)
            with tc.tile_wait_until(0.0055):
                load_tiles(16, 32)
            with tc.tile_wait_until(0.0065):
                load_tiles(32, 48)
            with tc.tile_wait_until(0.0075):
                load_tiles(48, 64)
            with tc.tile_wait_until(0.010):
                norm_tiles(16, 32)
                transpose_tiles(16, 32)
            with tc.tile_wait_until(0.016):
                norm_tiles(32, 48)
                transpose_tiles(32, 48)
            with tc.tile_wait_until(0.022):
                norm_tiles(48, 64)
                transpose_tiles(48, 64)
            with tc.tile_wait_until(0.028):
                # positive-pair dots: G_pos[p,b] = sum_d zn[p,b,d]*zn[p,b+32,d]
                nc.vector.tensor_mul(
                    pprod, zn_all[:, 0:NBLK, :], zn_all[:, 32 : 32 + NBLK, :]
                )
                nc.vector.reduce_sum(gpos, pprod, axis=AX.X)

            # ---- main exp stream ------------------------------------------
            # b=0 first at fine granularity, then (b, c) with c outer so znT
            # groups arrive in time.
            ct0 = 0
            for c, ctw in enumerate(CHUNKS_B0):
                emit_chunk(0, c, ct0, ctw)
                ct0 += ctw
            for c, ctw in enumerate(CHUNKS_BN):
                for b in range(1, NBLK):
                    emit_chunk(b, c, c * GRP, ctw)

            # ---- epilogue --------------------------------------------------
            sumexp = persist.tile([128, NBLK], F32)
            nc.vector.reduce_sum(sumexp, acc, axis=AX.X)      # [128,8,7] -> [128,8]
            S = persist.tile([128, NBLK], F32)
            nc.vector.tensor_scalar_add(S, sumexp, -1.0)      # drop diagonal (=1)
            lnS = persist.tile([128, NBLK], F32)
            nc.scalar.activation(lnS, S, AF.Ln, bias=b_zero)
            pterm = persist.tile([128, NBLK], F32)
            nc.vector.tensor_scalar(
                pterm, gpos, -INV_T, INV_T, ALU.mult, ALU.add  # 10 - 10*G_pos
            )
            contrib = persist.tile([128, NBLK], F32)
            nc.vector.tensor_add(contrib, lnS, pterm)
            total = persist.tile([128, 1], F32)
            nc.vector.reduce_sum(total, contrib, axis=AX.X)
            nc.sync.dma_start(out=out[:, :], in_=total)

    nc.compile()
    return nc


_NC = None


def _get_nc() -> bass.Bass:
    global _NC
    if _NC is None:
        _NC = _build()
    return _NC


def kernel(emb0: np.ndarray, emb1: np.ndarray) -> np.ndarray:
    z = np.concatenate(
        [np.asarray(emb0, np.float32), np.asarray(emb1, np.float32)], axis=0
    )
    in_maps = [
        {"z": np.ascontiguousarray(np.roll(z, -c * ROWS, axis=0))}
        for c in range(NCORES)
    ]
    res = bass_utils.run_bass_kernel_spmd(_get_nc(), in_maps, core_ids=list(range(NCORES)))
    total = sum(float(r["partial"].sum(dtype=np.float64)) for r in res.results)
    return np.asarray(np.float32(total / N))


# revision 8
# speedup vs baseline: 1.0578x; 1.0578x over previous
"""Trainium2 Bass kernel for nn_ContrastiveLoss (SimCLR-style, N=8192, D=128).

Sharding: rows of the NxN sim matrix split across 8 cores (1024 rows each).
Each core receives the full z = concat(emb0, emb1) ROTATED so its own rows
come first (np.roll(z, -core*1024, axis=0)).  With that rotation the positive
pair of local row l is local row l+4096 on every core -> one SPMD program,
no collectives.

Math (per row r, fixed max = 1.0 since cosine sim <= 1):
  S_r    = sum_j exp(10*G_rj - 10) - exp(10*G_rr - 10)   [G_rr = 1 => subtract 1]
  loss_r = (10 + ln S_r) - 10*G_pos
  loss   = mean_r(loss_r);  per-core output = [128,1] partial sums of loss_r.

Engine split (ACT is the bottleneck; everything else hides under it):
  ACT : one natural_log_exp table load (forced by a dummy Ln up front),
        exp(psum chunks) with accum_out row sums, tiny Ln at the end.
  PE  : only the G-matmuls (bf16).  Transposes moved to the DMA xbar.
  DVE : norms via mul+reduce + Newton rsqrt (fast-inverse-sqrt bit trick),
        normalize to bf16, positive-pair dot products, epilogue.
  DMA : sync queue: critical-path loads (tiles 0..15 in 4-tile slices),
        zn -> znT xbar transposes, output.  GpSimd software DGE: bulk loads
        (tiles 16..63), gated behind the critical chain by a dummy copy so
        their transfers / DVE chains don't contend with the prologue.

Row-block 0's first column chunks are 512 wide so the exp stream starts as
soon as the first 4-tile slice is normalized+transposed.
"""

import sys

sys.path.insert(0, "/opt/trn_rl_repo")

from contextlib import ExitStack

import numpy as np

import concourse.bass as bass
import concourse.bacc as bacc
import concourse.tile as tile
from concourse import mybir
from concourse import bass_utils

B = 4096
D = 128
N = 2 * B            # 8192 rows of z
NCORES = 8
ROWS = N // NCORES   # 1024 rows per core
NBLK = ROWS // 128   # 8 row-blocks per core
CHUNK = 2048         # psum tile width (4 banks)
SEG = 512            # matmul moving-operand width
NTILE = N // 128     # 64 partition-tiles of z
GRP = 16             # tiles per bulk DMA / norm group (groups 1..3)
SUB = 4              # tiles per slice in group 0
INV_T = 10.0         # 1/temperature
MAGIC = 0x5F3759DF
MAXC = 7             # b0: 4 subchunks + 3 big; b>=1: 4 big

F32 = mybir.dt.float32
BF16 = mybir.dt.bfloat16
U32 = mybir.dt.uint32
AX = mybir.AxisListType
AF = mybir.ActivationFunctionType
ALU = mybir.AluOpType


def _build() -> bass.Bass:
    nc = bacc.Bacc(None)
    z_in = nc.declare_dram_parameter("z", [N, D], F32, isOutput=False)
    out = nc.declare_dram_parameter("partial", [128, 1], F32, isOutput=True)

    z_re = z_in.rearrange("(n p) d -> p n d", p=128)  # row = n*128 + p

    with tile.TileContext(nc) as tc:
        with ExitStack() as ctx:
            persist = ctx.enter_context(tc.tile_pool(name="persist", bufs=1))
            junkp = ctx.enter_context(tc.tile_pool(name="junk", bufs=2))
            psum = ctx.enter_context(tc.tile_pool(name="psum", bufs=2, space="PSUM"))

            # non-Copy activations need bias as an SBUF AP
            b_zero = persist.tile([128, 1], F32)
            nc.vector.memset(b_zero, 0.0)
            b_one = persist.tile([128, 1], F32)
            nc.vector.memset(b_one, 1.0)
            b_neg10 = persist.tile([128, 1], F32)
            nc.vector.memset(b_neg10, -INV_T)
            magic = persist.tile([128, 1], U32)
            nc.vector.memset(magic, MAGIC)

            z_sb = persist.tile([128, NTILE, D], F32)
            sq = persist.tile([128, NTILE, D], F32)
            ss = persist.tile([128, NTILE], F32)       # sumsq -> clamped
            rn = persist.tile([128, NTILE], F32)       # 1/norm (Newton rsqrt)
            nt0 = persist.tile([128, NTILE], F32)      # newton temps
            nt1 = persist.tile([128, NTILE], F32)
            zn_all = persist.tile([128, NTILE, D], BF16)
            znT = persist.tile([128, NTILE, D], BF16)  # [d, tile, row%128]
            acc = persist.tile([128, NBLK, MAXC], F32)     # per-chunk exp sums
            pprod = persist.tile([128, NBLK, D], F32)      # pos-pair products
            gpos = persist.tile([128, NBLK], F32)          # G_pos per row
            lnjunk = persist.tile([128, 1], F32)
            gate = persist.tile([128, D], BF16)

            nc.vector.memset(acc, 0.0)
            # dummy Ln so the table pass picks natural_log_exp_and_others
            # (one load covers the whole kernel: exp stream + final Ln)
            nc.scalar.activation(lnjunk, b_one, AF.Ln, bias=b_zero)

            def norm_tiles(t0, t1):
                sl = slice(t0, t1)
                w = t1 - t0
                nc.vector.tensor_mul(sq[:, sl, :], z_sb[:, sl, :], z_sb[:, sl, :])
                nc.vector.reduce_sum(ss[:, sl], sq[:, sl, :], axis=AX.X)
                nc.vector.tensor_scalar_max(ss[:, sl], ss[:, sl], 1e-16)
                # y0 = bitcast(MAGIC - (bits(s) >> 1)); two Newton steps
                # (DVE int add saturates, so subtract from a const tile)
                s_u = ss[:, sl].bitcast(U32)
                y_u = rn[:, sl].bitcast(U32)
                t_u = nt0[:, sl].bitcast(U32)
                nc.vector.tensor_scalar(t_u, s_u, 1, None, ALU.logical_shift_right)
                nc.vector.tensor_sub(y_u, magic.broadcast_to((128, w)), t_u)
                for _ in range(2):
                    nc.vector.tensor_mul(nt0[:, sl], rn[:, sl], rn[:, sl])
                    nc.vector.tensor_mul(nt1[:, sl], nt0[:, sl], ss[:, sl])
                    nc.vector.tensor_scalar(
                        nt0[:, sl], nt1[:, sl], -0.5, 1.5, ALU.mult, ALU.add
                    )
                    nc.vector.tensor_mul(rn[:, sl], rn[:, sl], nt0[:, sl])
                nc.vector.tensor_mul(
                    zn_all[:, sl, :],
                    z_sb[:, sl, :],
                    rn[:, sl].broadcast_to((128, w, D)),
                )

            def transpose_tiles(t0, t1):
                sl = slice(t0, t1)
                nc.sync.dma_start_transpose(znT[:, sl, :], zn_all[:, sl, :])

            def emit_chunk(b, c, ct0, ctw):
                # G chunk: rows of block b vs column tiles [ct0, ct0+ctw)
                lhsT = znT[:, b, :]  # [128(d), 128 rows]
                pt = psum.tile([128, CHUNK], F32, tag="pp", name="pt")
                for s in range(ctw * D // SEG):
                    cs = ct0 + s * (SEG // D)
                    nc.tensor.matmul(
                        pt[:, s * SEG : (s + 1) * SEG],
                        lhsT,
                        znT[:, cs : cs + SEG // D, :],
                        start=True,
                        stop=True,
                    )
                ej = junkp.tile([128, CHUNK], BF16, tag="ej", name="ej")
                nc.scalar.activation(
                    ej[:, 0 : ctw * D], pt[:, 0 : ctw * D], AF.Exp,
                    scale=INV_T, bias=b_neg10,
                    accum_out=acc[:, b, c : c + 1],
                )

            # ---- prologue --------------------------------------------------
            # critical path: 4-tile slices of tiles 0..15 on the sync queue
            for s in range(4):
                sl = slice(s * SUB, (s + 1) * SUB)
                nc.sync.dma_start(out=z_sb[:, sl, :], in_=z_re[:, sl, :])
                norm_tiles(s * SUB, (s + 1) * SUB)
                transpose_tiles(s * SUB, (s + 1) * SUB)
            # bulk loads on gpsimd software DGE, gated behind the critical
            # chain (gate copy depends on slice 3's zn) so their transfers and
            # DVE chains don't compete with the prologue.
            nc.gpsimd.tensor_copy(gate, zn_all[:, 15, :])
            for g in range(1, 4):
                sl = slice(g * GRP, (g + 1) * GRP)
                nc.gpsimd.dma_start(out=z_sb[:, sl, :], in_=z_re[:, sl, :])
            norm_tiles(16, 32)
            transpose_tiles(16, 32)
            norm_tiles(32, 48)
            transpose_tiles(32, 48)
            # positive-pair dots: G_pos[p,b] = sum_d zn[p,b,d]*zn[p,b+32,d]
            nc.vector.tensor_mul(
                pprod, zn_all[:, 0:NBLK, :], zn_all[:, 32 : 32 + NBLK, :]
            )
            nc.vector.reduce_sum(gpos, pprod, axis=AX.X)
            norm_tiles(48, 64)
            transpose_tiles(48, 64)

            # ---- main exp stream ------------------------------------------
            # b0's four 512-wide subchunks first (each needs only one 4-tile
            # slice), then 2048-wide chunks with c outer; b0's remaining big
            # chunks ride along once their tile group is in use anyway.
            for s in range(4):
                emit_chunk(0, s, s * SUB, SUB)
            for c in range(4):
                for b in range(1, NBLK):
                    emit_chunk(b, c, c * GRP, GRP)
                if c >= 1:
                    emit_chunk(0, 3 + c, c * GRP, GRP)

            # ---- epilogue --------------------------------------------------
            sumexp = persist.tile([128, NBLK], F32)
            nc.vector.reduce_sum(sumexp, acc, axis=AX.X)      # [128,8,7] -> [128,8]
            S = persist.tile([128, NBLK], F32)
            nc.vector.tensor_scalar_add(S, sumexp, -1.0)      # drop diagonal (=1)
            lnS = persist.tile([128, NBLK], F32)
            nc.scalar.activation(lnS, S, AF.Ln, bias=b_zero)
            pterm = persist.tile([128, NBLK], F32)
            nc.vector.tensor_scalar(
                pterm, gpos, -INV_T, INV_T, ALU.mult, ALU.add  # 10 - 10*G_pos
            )
            contrib = persist.tile([128, NBLK], F32)
            nc.vector.tensor_add(contrib, lnS, pterm)
            total = persist.tile([128, 1], F32)
            nc.vector.reduce_sum(total, contrib, axis=AX.X)
            nc.sync.dma_start(out=out[:, :], in_=total)

    nc.compile()
    return nc


_NC = None


def _get_nc() -> bass.Bass:
    global _NC
    if _NC is None:
        _NC = _build()
    return _NC


def kernel(emb0: np.ndarray, emb1: np.ndarray) -> np.ndarray:
    z = np.concatenate(
        [np.asarray(emb0, np.float32), np.asarray(emb1, np.float32)], axis=0
    )
    in_maps = [
        {"z": np.ascontiguousarray(np.roll(z, -c * ROWS, axis=0))}
        for c in range(NCORES)
    ]
    res = bass_utils.run_bass_kernel_spmd(_get_nc(), in_maps, core_ids=list(range(NCORES)))
    total = sum(float(r["partial"].sum(dtype=np.float64)) for r in res.results)
    return np.asarray(np.float32(total / N))


# revision 9
# speedup vs baseline: 1.1054x; 1.0450x over previous
"""Trainium2 Bass kernel for nn_ContrastiveLoss (SimCLR-style, N=8192, D=128).

Sharding: rows of the NxN sim matrix split across 8 cores (1024 rows each).
Each core receives the full z = concat(emb0, emb1) ROTATED so its own rows
come first (np.roll(z, -core*1024, axis=0)).  With that rotation the positive
pair of local row l is local row l+4096 on every core -> one SPMD program,
no collectives.

Math (per row r, fixed max = 1.0 since cosine sim <= 1):
  S_r    = sum_j exp(10*G_rj - 10) - exp(10*G_rr - 10)   [G_rr = 1 => subtract 1]
  loss_r = (10 + ln S_r) - 10*G_pos
  loss   = mean_r(loss_r);  per-core output = [128,1] partial sums of loss_r.

Engine split (ACT is the bottleneck; everything else hides under it):
  ACT : one natural_log_exp table load (forced by a dummy Ln up front),
        exp(psum chunks) with accum_out row sums, tiny Ln at the end.
  PE  : only the G-matmuls (bf16).  Transposes moved to the DMA xbar.
  DVE : norms via mul+reduce + Newton rsqrt (fast-inverse-sqrt bit trick),
        normalize to bf16, positive-pair dot products, epilogue.
  DMA : sync queue: critical-path loads (tiles 0..15 in 4-tile slices),
        zn -> znT xbar transposes, output.  GpSimd software DGE: bulk loads
        (tiles 16..63), gated behind the critical chain by a dummy copy so
        their transfers / DVE chains don't contend with the prologue.

Row-block 0's first column chunks are 512 wide so the exp stream starts as
soon as the first 4-tile slice is normalized+transposed.
"""

import sys

sys.path.insert(0, "/opt/trn_rl_repo")

from contextlib import ExitStack

import numpy as np

import concourse.bass as bass
import concourse.bacc as bacc
import concourse.tile as tile
from concourse import mybir
from concourse import bass_utils

B = 4096
D = 128
N = 2 * B            # 8192 rows of z
NCORES = 8
ROWS = N // NCORES   # 1024 rows per core
NBLK = ROWS // 128   # 8 row-blocks per core
CHUNK = 2048         # psum tile width (4 banks)
SEG = 512            # matmul moving-operand width
NTILE = N // 128     # 64 partition-tiles of z
GRP = 16             # tiles per bulk DMA / norm group (groups 1..3)
SUB = 4              # tiles per slice in group 0
INV_T = 10.0         # 1/temperature
MAGIC = 0x5F3759DF
MAXC = 7             # b0: 4 subchunks + 3 big; b>=1: 4 big

F32 = mybir.dt.float32
BF16 = mybir.dt.bfloat16
U32 = mybir.dt.uint32
AX = mybir.AxisListType
AF = mybir.ActivationFunctionType
ALU = mybir.AluOpType


def _build() -> bass.Bass:
    nc = bacc.Bacc(None)
    z_in = nc.declare_dram_parameter("z", [N, D], F32, isOutput=False)
    out = nc.declare_dram_parameter("partial", [128, 1], F32, isOutput=True)

    z_re = z_in.rearrange("(n p) d -> p n d", p=128)  # row = n*128 + p

    with tile.TileContext(nc) as tc:
        with ExitStack() as ctx:
            persist = ctx.enter_context(tc.tile_pool(name="persist", bufs=1))
            junkp = ctx.enter_context(tc.tile_pool(name="junk", bufs=2))
            psum = ctx.enter_context(tc.tile_pool(name="psum", bufs=2, space="PSUM"))

            # non-Copy activations need bias as an SBUF AP
            b_zero = persist.tile([128, 1], F32)
            nc.vector.memset(b_zero, 0.0)
            b_one = persist.tile([128, 1], F32)
            nc.vector.memset(b_one, 1.0)
            b_neg10 = persist.tile([128, 1], F32)
            nc.vector.memset(b_neg10, -INV_T)
            magic = persist.tile([128, 1], U32)
            nc.vector.memset(magic, MAGIC)

            z_sb = persist.tile([128, NTILE, D], F32)
            sq = persist.tile([128, NTILE, D], F32)
            ss = persist.tile([128, NTILE], F32)       # sumsq -> clamped
            rn = persist.tile([128, NTILE], F32)       # 1/norm (Newton rsqrt)
            nt0 = persist.tile([128, NTILE], F32)      # newton temps
            nt1 = persist.tile([128, NTILE], F32)
            zn_all = persist.tile([128, NTILE, D], BF16)
            znT = persist.tile([128, NTILE, D], BF16)  # [d, tile, row%128]
            acc = persist.tile([128, NBLK, MAXC], F32)     # per-chunk exp sums
            pprod = persist.tile([128, NBLK, D], F32)      # pos-pair products
            gpos = persist.tile([128, NBLK], F32)          # G_pos per row
            lnjunk = persist.tile([128, 1], F32)
            gate = persist.tile([128, D], BF16)

            nc.vector.memset(acc, 0.0)
            # dummy Ln so the table pass picks natural_log_exp_and_others
            # (one load covers the whole kernel: exp stream + final Ln)
            nc.scalar.activation(lnjunk, b_one, AF.Ln, bias=b_zero)

            def norm_tiles(t0, t1):
                sl = slice(t0, t1)
                w = t1 - t0
                nc.vector.tensor_mul(sq[:, sl, :], z_sb[:, sl, :], z_sb[:, sl, :])
                nc.vector.reduce_sum(ss[:, sl], sq[:, sl, :], axis=AX.X)
                nc.vector.tensor_scalar_max(ss[:, sl], ss[:, sl], 1e-16)
                # y0 = bitcast(MAGIC - (bits(s) >> 1)); two Newton steps
                # (DVE int add saturates, so subtract from a const tile)
                s_u = ss[:, sl].bitcast(U32)
                y_u = rn[:, sl].bitcast(U32)
                t_u = nt0[:, sl].bitcast(U32)
                nc.vector.tensor_scalar(t_u, s_u, 1, None, ALU.logical_shift_right)
                nc.vector.tensor_sub(y_u, magic.broadcast_to((128, w)), t_u)
                for _ in range(2):
                    nc.vector.tensor_mul(nt0[:, sl], rn[:, sl], rn[:, sl])
                    nc.vector.tensor_mul(nt1[:, sl], nt0[:, sl], ss[:, sl])
                    nc.vector.tensor_scalar(
                        nt0[:, sl], nt1[:, sl], -0.5, 1.5, ALU.mult, ALU.add
                    )
                    nc.vector.tensor_mul(rn[:, sl], rn[:, sl], nt0[:, sl])
                nc.vector.tensor_mul(
                    zn_all[:, sl, :],
                    z_sb[:, sl, :],
                    rn[:, sl].broadcast_to((128, w, D)),
                )

            def transpose_tiles(t0, t1):
                sl = slice(t0, t1)
                nc.sync.dma_start_transpose(znT[:, sl, :], zn_all[:, sl, :])

            def emit_chunk(b, c, ct0, ctw):
                # G chunk: rows of block b vs column tiles [ct0, ct0+ctw)
                lhsT = znT[:, b, :]  # [128(d), 128 rows]
                pt = psum.tile([128, CHUNK], F32, tag="pp", name="pt")
                for s in range(ctw * D // SEG):
                    cs = ct0 + s * (SEG // D)
                    nc.tensor.matmul(
                        pt[:, s * SEG : (s + 1) * SEG],
                        lhsT,
                        znT[:, cs : cs + SEG // D, :],
                        start=True,
                        stop=True,
                    )
                ej = junkp.tile([128, CHUNK], BF16, tag="ej", name="ej")
                nc.scalar.activation(
                    ej[:, 0 : ctw * D], pt[:, 0 : ctw * D], AF.Exp,
                    scale=INV_T, bias=b_neg10,
                    accum_out=acc[:, b, c : c + 1],
                )

            # ---- prologue --------------------------------------------------
            # critical path: 4-tile slices of tiles 0..15, posted first so
            # their ring packets precede the bulk loads (per-ring FIFO).
            for s in range(4):
                sl = slice(s * SUB, (s + 1) * SUB)
                nc.sync.dma_start(out=z_sb[:, sl, :], in_=z_re[:, sl, :])
                norm_tiles(s * SUB, (s + 1) * SUB)
                transpose_tiles(s * SUB, (s + 1) * SUB)
            # bulk loads (tiles 16..63); ready at t=0 but lower priority than
            # the sub loads, so the scheduler posts them after.
            for g in range(1, 4):
                sl = slice(g * GRP, (g + 1) * GRP)
                nc.sync.dma_start(out=z_sb[:, sl, :], in_=z_re[:, sl, :])
            norm_tiles(16, 32)
            transpose_tiles(16, 32)
            norm_tiles(32, 48)
            transpose_tiles(32, 48)
            # positive-pair dots: G_pos[p,b] = sum_d zn[p,b,d]*zn[p,b+32,d]
            nc.vector.tensor_mul(
                pprod, zn_all[:, 0:NBLK, :], zn_all[:, 32 : 32 + NBLK, :]
            )
            nc.vector.reduce_sum(gpos, pprod, axis=AX.X)
            norm_tiles(48, 64)
            transpose_tiles(48, 64)

            # ---- main exp stream ------------------------------------------
            # b0's four 512-wide subchunks first (each needs only one 4-tile
            # slice), then 2048-wide chunks with c outer; b0's remaining big
            # chunks ride along once their tile group is in use anyway.
            for s in range(4):
                emit_chunk(0, s, s * SUB, SUB)
            for c in range(4):
                for b in range(1, NBLK):
                    emit_chunk(b, c, c * GRP, GRP)
                if c >= 1:
                    emit_chunk(0, 3 + c, c * GRP, GRP)

            # ---- epilogue --------------------------------------------------
            sumexp = persist.tile([128, NBLK], F32)
            nc.vector.reduce_sum(sumexp, acc, axis=AX.X)      # [128,8,7] -> [128,8]
            S = persist.tile([128, NBLK], F32)
            nc.vector.tensor_scalar_add(S, sumexp, -1.0)      # drop diagonal (=1)
            lnS = persist.tile([128, NBLK], F32)
            nc.scalar.activation(lnS, S, AF.Ln, bias=b_zero)
            pterm = persist.tile([128, NBLK], F32)
            nc.vector.tensor_scalar(
                pterm, gpos, -INV_T, INV_T, ALU.mult, ALU.add  # 10 - 10*G_pos
            )
            contrib = persist.tile([128, NBLK], F32)
            nc.vector.tensor_add(contrib, lnS, pterm)
            total = persist.tile([128, 1], F32)
            nc.vector.reduce_sum(total, contrib, axis=AX.X)
            nc.sync.dma_start(out=out[:, :], in_=total)

    nc.compile()
    return nc


_NC = None


def _get_nc() -> bass.Bass:
    global _NC
    if _NC is None:
        _NC = _build()
    return _NC


def kernel(emb0: np.ndarray, emb1: np.ndarray) -> np.ndarray:
    z = np.concatenate(
        [np.asarray(emb0, np.float32), np.asarray(emb1, np.float32)], axis=0
    )
    in_maps = [
        {"z": np.ascontiguousarray(np.roll(z, -c * ROWS, axis=0))}
        for c in range(NCORES)
    ]
    res = bass_utils.run_bass_kernel_spmd(_get_nc(), in_maps, core_ids=list(range(NCORES)))
    total = sum(float(r["partial"].sum(dtype=np.float64)) for r in res.results)
    return np.asarray(np.float32(total / N))


# revision 10
# speedup vs baseline: 1.2419x; 1.1235x over previous
"""Trainium2 Bass kernel for nn_ContrastiveLoss (SimCLR-style, N=8192, D=128).

Sharding: rows of the NxN sim matrix split across 8 cores (1024 rows each).
Each core receives the full z = concat(emb0, emb1) ROTATED so its own rows
come first (np.roll(z, -core*1024, axis=0)).  With that rotation the positive
pair of local row l is local row l+4096 on every core -> one SPMD program,
no collectives.

Math (per row r, fixed max = 1.0 since cosine sim <= 1):
  S_r    = sum_j exp(10*G_rj - 10) - exp(10*G_rr - 10)   [G_rr = 1 => subtract 1]
  loss_r = (10 + ln S_r) - 10*G_pos
  loss   = mean_r(loss_r)
The kernel outputs per-chunk exp row sums (acc) and G_pos per row; the final
ln / subtract / mean run on host (8192 lns in numpy are free and this removes
an activation-table switch plus the whole device epilogue from the tail).

Engine split (ACT is the bottleneck; everything else hides under it):
  ACT : one exp table load, exp(psum chunks) with accum_out row sums.
  PE  : only the G-matmuls (bf16).  Transposes moved to the DMA xbar.
  DVE : norms via mul+reduce + Newton rsqrt (fast-inverse-sqrt bit trick),
        normalize to bf16, positive-pair dot products.
  DMA : sync queue: loads (critical 8-tile slices first, then bulk),
        zn -> znT xbar transposes, outputs.

Dependencies are tracked per-engine as monotonic clocks, so the critical
chain (tiles 0..8 -> first exp) is emitted first on each engine and the
fine first slices keep everything else out of its way.
"""

import sys

sys.path.insert(0, "/opt/trn_rl_repo")

from contextlib import ExitStack

import numpy as np

import concourse.bass as bass
import concourse.bacc as bacc
import concourse.tile as tile
from concourse import mybir
from concourse import bass_utils

B = 4096
D = 128
N = 2 * B            # 8192 rows of z
NCORES = 8
ROWS = N // NCORES   # 1024 rows per core
NBLK = ROWS // 128   # 8 row-blocks per core
CHUNK = 2048         # psum tile width (4 banks)
SEG = 512            # matmul moving-operand width
NTILE = N // 128     # 64 partition-tiles of z
INV_T = 10.0         # 1/temperature
MAGIC = 0x5F3759DF
MAXC = 5             # b0: 2x1024 + 3x2048; b>=1: 4x2048

F32 = mybir.dt.float32
BF16 = mybir.dt.bfloat16
U32 = mybir.dt.uint32
AX = mybir.AxisListType
AF = mybir.ActivationFunctionType
ALU = mybir.AluOpType


def _build() -> bass.Bass:
    nc = bacc.Bacc(None)
    z_in = nc.declare_dram_parameter("z", [N, D], F32, isOutput=False)
    acc_out = nc.declare_dram_parameter("acc", [128, NBLK, MAXC], F32, isOutput=True)
    gpos_out = nc.declare_dram_parameter("gpos", [128, NBLK], F32, isOutput=True)

    z_re = z_in.rearrange("(n p) d -> p n d", p=128)  # row = n*128 + p

    with tile.TileContext(nc) as tc:
        with ExitStack() as ctx:
            persist = ctx.enter_context(tc.tile_pool(name="persist", bufs=1))
            junkp = ctx.enter_context(tc.tile_pool(name="junk", bufs=2))
            psum = ctx.enter_context(tc.tile_pool(name="psum", bufs=2, space="PSUM"))

            # non-Copy activations need bias as an SBUF AP
            b_neg10 = persist.tile([128, 1], F32)
            nc.vector.memset(b_neg10, -INV_T)
            magic = persist.tile([128, 1], U32)
            nc.vector.memset(magic, MAGIC)

            z_sb = persist.tile([128, NTILE, D], F32)
            sq = persist.tile([128, NTILE, D], F32)
            ss = persist.tile([128, NTILE], F32)       # sumsq -> clamped
            rn = persist.tile([128, NTILE], F32)       # 1/norm (Newton rsqrt)
            nt0 = persist.tile([128, NTILE], F32)      # newton temps
            nt1 = persist.tile([128, NTILE], F32)
            zn_all = persist.tile([128, NTILE, D], BF16)
            znT = persist.tile([128, NTILE, D], BF16)  # [d, tile, row%128]
            acc = persist.tile([128, NBLK, MAXC], F32)     # per-chunk exp sums
            pprod = persist.tile([128, NBLK, D], F32)      # pos-pair products
            gpos = persist.tile([128, NBLK], F32)          # G_pos per row

            nc.vector.memset(acc, 0.0)

            def load_tiles(t0, t1):
                sl = slice(t0, t1)
                nc.sync.dma_start(out=z_sb[:, sl, :], in_=z_re[:, sl, :])

            def norm_tiles(t0, t1):
                sl = slice(t0, t1)
                w = t1 - t0
                nc.vector.tensor_mul(sq[:, sl, :], z_sb[:, sl, :], z_sb[:, sl, :])
                nc.vector.reduce_sum(ss[:, sl], sq[:, sl, :], axis=AX.X)
                nc.vector.tensor_scalar_max(ss[:, sl], ss[:, sl], 1e-16)
                # y0 = bitcast(MAGIC - (bits(s) >> 1)); two Newton steps
                # (DVE int add saturates, so subtract from a const tile)
                s_u = ss[:, sl].bitcast(U32)
                y_u = rn[:, sl].bitcast(U32)
                t_u = nt0[:, sl].bitcast(U32)
                nc.vector.tensor_scalar(t_u, s_u, 1, None, ALU.logical_shift_right)
                nc.vector.tensor_sub(y_u, magic.broadcast_to((128, w)), t_u)
                for _ in range(2):
                    nc.vector.tensor_mul(nt0[:, sl], rn[:, sl], rn[:, sl])
                    nc.vector.tensor_mul(nt1[:, sl], nt0[:, sl], ss[:, sl])
                    nc.vector.tensor_scalar(
                        nt0[:, sl], nt1[:, sl], -0.5, 1.5, ALU.mult, ALU.add
                    )
                    nc.vector.tensor_mul(rn[:, sl], rn[:, sl], nt0[:, sl])
                nc.vector.tensor_mul(
                    zn_all[:, sl, :],
                    z_sb[:, sl, :],
                    rn[:, sl].broadcast_to((128, w, D)),
                )

            def transpose_tiles(t0, t1):
                sl = slice(t0, t1)
                nc.sync.dma_start_transpose(znT[:, sl, :], zn_all[:, sl, :])

            def emit_chunk(b, c, ct0, ctw):
                # G chunk: rows of block b vs column tiles [ct0, ct0+ctw)
                lhsT = znT[:, b, :]  # [128(d), 128 rows]
                pt = psum.tile([128, CHUNK], F32, tag="pp", name="pt")
                for s in range(ctw * D // SEG):
                    cs = ct0 + s * (SEG // D)
                    nc.tensor.matmul(
                        pt[:, s * SEG : (s + 1) * SEG],
                        lhsT,
                        znT[:, cs : cs + SEG // D, :],
                        start=True,
                        stop=True,
                    )
                ej = junkp.tile([128, CHUNK], BF16, tag="ej", name="ej")
                nc.scalar.activation(
                    ej[:, 0 : ctw * D], pt[:, 0 : ctw * D], AF.Exp,
                    scale=INV_T, bias=b_neg10,
                    accum_out=acc[:, b, c : c + 1],
                )

            # ---- prologue --------------------------------------------------
            # critical path: two 8-tile slices, then bulk; slice loads are
            # posted first so their ring packets precede the bulk transfers.
            load_tiles(0, 8)
            norm_tiles(0, 8)
            transpose_tiles(0, 8)
            load_tiles(8, 16)
            norm_tiles(8, 16)
            transpose_tiles(8, 16)
            load_tiles(16, 40)
            load_tiles(40, 64)
            norm_tiles(16, 40)
            transpose_tiles(16, 40)
            # positive-pair dots: G_pos[p,b] = sum_d zn[p,b,d]*zn[p,b+32,d]
            nc.vector.tensor_mul(
                pprod, zn_all[:, 0:NBLK, :], zn_all[:, 32 : 32 + NBLK, :]
            )
            nc.vector.reduce_sum(gpos, pprod, axis=AX.X)
            nc.sync.dma_start(out=gpos_out[:, :], in_=gpos)
            norm_tiles(40, 64)
            transpose_tiles(40, 64)

            # ---- main exp stream ------------------------------------------
            # b0's two 1024-wide chunks first (each needs only one 8-tile
            # slice), then 2048-wide chunks with c outer; b0's remaining big
            # chunks ride along once their tile group is loaded anyway.
            emit_chunk(0, 0, 0, 8)
            emit_chunk(0, 1, 8, 8)
            for c in range(4):
                for b in range(1, NBLK):
                    emit_chunk(b, c, c * 16, 16)
                if c >= 1:
                    emit_chunk(0, 1 + c, c * 16, 16)

            nc.sync.dma_start(out=acc_out[:, :, :], in_=acc)

    nc.compile()
    return nc


_NC = None


def _get_nc() -> bass.Bass:
    global _NC
    if _NC is None:
        _NC = _build()
    return _NC


def kernel(emb0: np.ndarray, emb1: np.ndarray) -> np.ndarray:
    z = np.concatenate(
        [np.asarray(emb0, np.float32), np.asarray(emb1, np.float32)], axis=0
    )
    in_maps = [
        {"z": np.ascontiguousarray(np.roll(z, -c * ROWS, axis=0))}
        for c in range(NCORES)
    ]
    res = bass_utils.run_bass_kernel_spmd(_get_nc(), in_maps, core_ids=list(range(NCORES)))
    total = 0.0
    for r in res.results:
        S = r["acc"].astype(np.float64).sum(axis=2) - 1.0   # [128, NBLK]
        gp = r["gpos"].astype(np.float64)
        total += float((np.log(S) - INV_T * gp + INV_T).sum())
    return np.asarray(np.float32(total / N))
